# revision 27
# baseline (speedup 1.0000x reference)
"""MultiHeadDecoder (moe_routing) Trainium2 kernel, v17.

Expert-parallel: each of 8 cores owns one head. Host groups samples by
head, pads to capacity C (multiple of 8), ships everything bf16 (PSUM
accumulates f32; tolerance 2e-2 vs bf16 wire error ~4e-3; fp8/DoubleRow
was simulated exactly and FAILS the gate at 3.4e-2).

Both stages keep weights stationary in the PE and stream sample columns:
  stage A:  ht[hc][hid,s]  = relu(sum_k W1[k,hc]^T @ X^T[k][:,s] + b1)
  stage B:  outT[of][of,s] = sum_hc W2[of,hc]^T @ ht[hc][:,s] + b2
Output is transposed ([out_feature, sample]); host untransposes.

Timing model (measured): fixed NEFF preamble ~7us; dynamic DMA rings'
first packet ~8.1-8.7/9.3-9.4/10.4us (sync/scalar/gpsimd); dense DRAM
input regions move ~8-10ns/packet vs ~15 strided; ~0.9us completion-sem
latency after a DMA's last packet; teardown (checks + barrier +
semaphore zero-storm) ~2.7-3.6us, scaling weakly with DMA count.
PE HAM: 1.2GHz cold, un-throttles after ~3.4-6.8us of sustained PE
busy (free-running window: the dominant +-1.5us run-to-run variance);
warm MM gap = N/2.4 + 2.5ns (112ns at N=264); any PE-idle hole before
the first xin-gated matmul resets the window (measured +2.6us).

Schedule: warmup matmuls bridge program start to xin arrival with no
hole.  xin ships as two dense halves (k0+W1k0+biases on sync, k1+W1k1
on scalar) so relu never gates on the tail; w2 as eight dense 2-of-tile
blocks across the rings, each landing >=2us before its stage-B need.
Stage A is k-major with k1 trailing one hc behind k0, so hc0's psum
closes right after the k1 half lands and the relu chain (g0->DVE,
g1->ACT, per-hc, ~2us) starts early.  One 8-bank PSUM pool serves
warmup + stage A + stage B: stage B holds 4 of-tiles of headroom, so
the psum-reuse convoy (of n+2 gated on of n's bias) never binds.
of0/of1 interleave hc-major to absorb the relu chain once.  Outputs
ride the sync ring as of-pairs (keeps it hot); the endgame fans out:
of14 on gpsimd, of15's g-halves on scalar+sync, so the final transfers
issue and stream in parallel.
"""

import numpy as np

import concourse.bass as bass
import concourse.mybir as mybir
from concourse import bacc
from concourse.tile import TileContext
from concourse.bass_utils import run_bass_kernel_spmd

IN_F, HID, OUT_F, N_HEADS, BATCH = 256, 512, 2048, 8, 4096
N_CORES = 8
P = 128
KI = IN_F // P      # 2 input-feature chunks
HC = HID // P       # 4 hidden chunks
OF = OUT_F // P     # 16 output-feature tiles
NB = HC + OF        # bias cols (b1: 4, b2: 16)

f32 = mybir.dt.float32
bf16 = mybir.dt.bfloat16

try:
    from ml_dtypes import bfloat16 as np_bf16
except ImportError:
    import jax.numpy as jnp
    np_bf16 = jnp.bfloat16

_NC_CACHE: dict = {}

# Warmups must bridge the PE seamlessly from program start (~6.8-7.4us)
# to the first xin-gated matmul (~10.3-11us): any PE-idle hole before the
# real work resets the HAM activity window and the whole of stage A runs
# at 1.2GHz (measured +2.6us on a WARM_PRE=10 run with a 1.2us hole).
WARM_PRE = 16    # 264-col warmups bridging program start to xin arrival


def build_nc(C: int):
    """Per-core Bass program for sample capacity C (multiple of 8)."""
    G = C // 2
    assert G <= 512
    H0 = C + HID + NB    # cols in xin half 0: X^T k0 | W1 k0 | b1 | b2
    H1 = C + HID         # cols in xin half 1: X^T k1 | W1 k1
    NIN = H0 + H1
    W2C = HC * P         # 512 w2 cols per of-tile

    nc = bacc.Bacc("TRN2", target_bir_lowering=False, debug=False,
                   num_devices=N_CORES)
    # Inputs are laid out so every DMA reads a fully dense DRAM region
    # (consecutive partition rows adjacent), letting the DMA engines
    # coalesce packets instead of moving one strided line per packet.
    xin0 = nc.dram_tensor("xin0", [P, H0], bf16, kind="ExternalInput")
    xin1 = nc.dram_tensor("xin1", [P, H1], bf16, kind="ExternalInput")
    w2 = nc.dram_tensor("w2", [8, P, 2 * W2C], bf16, kind="ExternalInput")
    outT = nc.dram_tensor("outT", [P, 14 * C], bf16, kind="ExternalOutput")
    # of14/of15 get their own contiguous [P*C] regions: consecutive
    # partitions are adjacent in DRAM, so the endgame DMAs coalesce into
    # fat packets (the strided [P, OF*C] layout moves 1KB lines).
    outT2 = nc.dram_tensor("outT2", [2, P, C], bf16, kind="ExternalOutput")

    relu_f = mybir.ActivationFunctionType.Relu
    ident = mybir.ActivationFunctionType.Identity
    op_add = mybir.AluOpType.add
    op_max = mybir.AluOpType.max

    with TileContext(nc) as tc:
        with (
            tc.tile_pool(name="const", bufs=1) as const,
            tc.tile_pool(name="psum", bufs=8, space="PSUM") as psum,
            tc.tile_pool(name="outp", bufs=3) as outp,
            tc.tile_pool(name="outt", bufs=2) as outt,
        ):
            # Warmup matmuls on an uninitialized tile (values irrelevant).
            wsrc = const.tile([P, max(264, G)], bf16, tag="warm")
            nc.gpsimd.memset(wsrc[:, :1], 0.0)
            wps = psum.tile([P, G], f32, tag="ps", name="warmps")
            for i in range(WARM_PRE):
                nc.tensor.matmul(wps[:], lhsT=wsrc[:, :P],
                                 rhs=wsrc[:, :G], start=True, stop=True)

            # --- input DMAs ---
            xs = const.tile([P, NIN], bf16, tag="xin")
            w2s = const.tile([P, OF * W2C], bf16, tag="w2s")
            HB = 2 * W2C

            nc.sync.dma_start(xs[:, :H0], xin0[:])
            nc.scalar.dma_start(xs[:, H0:], xin1[:])
            # w2 ships as 2-of-tile blocks, each with its own completion
            # semaphore, spread over the three rings so every block lands
            # well before its stage-B need (a fused of4-7 block was
            # observed completing only ~18us on a cold-HAM run, starving
            # of4's ldweights for ~0.9us).
            w2_order = [(nc.gpsimd, 0), (nc.sync, 1), (nc.scalar, 2),
                        (nc.scalar, 3), (nc.gpsimd, 4), (nc.gpsimd, 5),
                        (nc.sync, 6), (nc.sync, 7)]
            for eng, blk in w2_order:
                eng.dma_start(w2s[:, blk * HB:(blk + 1) * HB], w2[blk])

            def xt_cols(k, g):
                base = (0 if k == 0 else H0) + g * G
                return xs[:, base:base + G]

            def w1_tile(k, hc):
                base = (0 if k == 0 else H0) + C + hc * P
                return xs[:, base:base + P]

            def w2_tile(of, hc):
                b = of * W2C + hc * P
                return w2s[:, b:b + P]
            # (w2s columns are of-major: of*512 + hc*128 + oc)

            # biases ship bf16 at the tail of xin half 0; convert to f32
            bconv = const.tile([P, NB], f32, tag="bconv")
            nc.gpsimd.tensor_scalar_add(bconv[:], xs[:, C + HID:H0], 0.0)
            b1_s = bconv[:, 0:HC]
            b2_s = bconv[:, HC:NB]

            # --- stage A: ht[hc] = relu(X @ W1 + b1)^T ---
            # Full k-major (8 concurrent accumulators, 7 pool slots + the
            # warmup slot): all k0 matmuls run before the k1 xin half
            # lands.  relu fans g0->DVE / g1->ACT in hc production order
            # (gpsimd cannot read PSUM).
            hts = [const.tile([P, C], bf16, tag=f"ht{hc}", name=f"ht{hc}")
                   for hc in range(HC)]
            pssA = {(hc, g): psum.tile([P, G], f32, tag="ps",
                                       name=f"psA{hc}_{g}")
                    for hc in range(HC) for g in range(2)}
            # k1 trails the k0 stream by one hc so hc0's accumulation
            # finishes (and its relu starts) ~0.5us after the k1 xin
            # half lands, instead of after the whole k0 sweep.
            for k, hc in [(0, 0), (0, 1), (1, 0), (0, 2),
                          (1, 1), (0, 3), (1, 2), (1, 3)]:
                for g in range(2):
                    nc.tensor.matmul(
                        pssA[hc, g][:],
                        lhsT=w1_tile(k, hc),
                        rhs=xt_cols(k, g),
                        start=(k == 0), stop=(k == KI - 1),
                    )
            for hc in range(HC):
                nc.vector.tensor_scalar(
                    hts[hc][:, 0:G], pssA[hc, 0][:],
                    b1_s[:, hc:hc + 1], 0.0, op_add, op_max,
                )
                nc.scalar.activation(hts[hc][:, G:C], pssA[hc, 1][:],
                                     relu_f, bias=b1_s[:, hc:hc + 1])

            # --- stage B: outT[of] = (H @ W2 + b2)^T, bf16 ---
            # of0/of1 interleave hc-major: both are paced by the same relu
            # chain, so running them in lockstep absorbs the chain once.
            # Outputs: of-pairs (and two final singles) all on the sync
            # ring so it stays hot into the endgame.
            def bias_out(pss, of, dst_t, off):
                for g in range(2):
                    dst = dst_t[:, off + g * G:off + (g + 1) * G]
                    if g == 1:
                        nc.scalar.activation(dst, pss[g][:], ident,
                                             bias=b2_s[:, of:of + 1])
                    else:
                        nc.vector.tensor_scalar_add(dst, pss[g][:],
                                                    b2_s[:, of:of + 1])

            ot = outp.tile([P, 2 * C], bf16, tag="op")
            pss01 = [[psum.tile([P, G], f32, tag="ps", name=f"psB{of}_{g}")
                      for g in range(2)] for of in range(2)]
            for hc in range(HC):
                for of in range(2):
                    for g in range(2):
                        nc.tensor.matmul(
                            pss01[of][g][:],
                            lhsT=w2_tile(of, hc),
                            rhs=hts[hc][:, g * G:(g + 1) * G],
                            start=(hc == 0), stop=(hc == HC - 1),
                        )
            bias_out(pss01[0], 0, ot, 0)
            bias_out(pss01[1], 1, ot, C)
            nc.sync.dma_start(outT[:, 0:2 * C], ot[:])

            o14 = o15 = None
            for of in range(2, OF):
                pss = [psum.tile([P, G], f32, tag="ps", name=f"psB{of}_{g}")
                       for g in range(2)]
                for hc in range(HC):
                    for g in range(2):
                        nc.tensor.matmul(
                            pss[g][:],
                            lhsT=w2_tile(of, hc),
                            rhs=hts[hc][:, g * G:(g + 1) * G],
                            start=(hc == 0), stop=(hc == HC - 1),
                        )
                if of < 14:
                    if of % 2 == 0:
                        ot = outp.tile([P, 2 * C], bf16, tag="op")
                    dst_t, off = ot, (of % 2) * C
                elif of == 14:
                    o14 = outt.tile([P, C], bf16, tag="o14")
                    dst_t, off = o14, 0
                else:
                    o15 = outt.tile([P, C], bf16, tag="o15")
                    dst_t, off = o15, 0
                bias_out(pss, of, dst_t, off)
                if of < 14 and of % 2 == 1:
                    pair = of // 2
                    nc.sync.dma_start(
                        outT[:, 2 * pair * C:(2 * pair + 2) * C], ot[:])
                elif of == 14:
                    # Endgame drain fans out over three engines/rings so
                    # the final transfers issue and stream in parallel
                    # instead of serializing behind one engine's ~0.6us
                    # dma_start cost each.
                    nc.gpsimd.dma_start(outT2[0], o14[:])
                elif of == 15:
                    nc.scalar.dma_start(outT2[1][:, :G], o15[:, :G])
                    nc.sync.dma_start(outT2[1][:, G:], o15[:, G:])

    nc.compile()
    return nc


def kernel(X, X_head_idx, W1, b1, W2, b2):
    X = np.ascontiguousarray(np.asarray(X, dtype=np.float32))
    idx = np.asarray(X_head_idx).astype(np.int64)
    W1 = np.asarray(W1, dtype=np.float32)
    b1 = np.asarray(b1, dtype=np.float32)
    W2 = np.asarray(W2, dtype=np.float32)
    b2 = np.asarray(b2, dtype=np.float32)

    batch = X.shape[0]
    counts = np.bincount(idx, minlength=N_HEADS)
    order = np.argsort(idx, kind="stable")
    positions = np.split(order, np.cumsum(counts)[:-1])

    C = max(64, int(-(-int(counts.max()) // 8)) * 8)
    if C not in _NC_CACHE:
        _NC_CACHE[C] = build_nc(C)
    nc = _NC_CACHE[C]

    H0 = C + HID + NB
    H1 = C + HID
    NIN = H0 + H1

    in_maps = []
    for h in range(N_HEADS):
        pos = positions[h]
        cnt = len(pos)
        xinf = np.zeros((P, NIN), dtype=np.float32)
        w1r = W1[h].reshape(KI, P, HID)
        if cnt:
            xk = X[pos].T.reshape(KI, P, cnt)  # [k, p, s]
            xinf[:, 0:cnt] = xk[0]
            xinf[:, H0:H0 + cnt] = xk[1]
        xinf[:, C:C + HID] = w1r[0]
        xinf[:, H0 + C:H0 + C + HID] = w1r[1]
        xinf[:, C + HID:C + HID + HC] = b1[h].reshape(HC, P).T
        xinf[:, C + HID + HC:H0] = b2[h].reshape(OF, P).T
        # w2 packed: [p, of*512 + hc*128 + oc] = W2[hc*128+p, of*128+oc],
        # then split into 8 dense [P, 1024]-col blocks of 2 of-tiles.
        w2r = W2[h].reshape(HC, P, OF, P)              # [hc, p, of, oc]
        w2p = np.ascontiguousarray(np.transpose(w2r, (1, 2, 0, 3)))
        w2p = w2p.reshape(P, 8, 2 * HC * P).transpose(1, 0, 2)
        in_maps.append({
            "xin0": xinf[:, :H0].astype(np_bf16),
            "xin1": xinf[:, H0:].astype(np_bf16),
            "w2": np.ascontiguousarray(w2p).astype(np_bf16),
        })

    try:
        res = run_bass_kernel_spmd(nc, in_maps, list(range(N_CORES)))
    except Exception:
        res = run_bass_kernel_spmd(nc, in_maps, list(range(N_CORES)))

    out = np.empty((batch, OUT_F), dtype=np.float32)
    for h in range(N_HEADS):
        pos = positions[h]
        cnt = len(pos)
        if cnt:
            o = np.asarray(res.results[h]["outT"]).astype(np.float32)
            o = o.reshape(P, 14, C)                    # [p, of, s]
            o = np.transpose(o, (2, 1, 0))             # [s, of, p]
            out[pos, :14 * P] = o.reshape(C, 14 * P)[:cnt]
            o2 = np.asarray(res.results[h]["outT2"]).astype(np.float32)
            o2 = np.transpose(o2, (2, 0, 1))           # [s, of, p]
            out[pos, 14 * P:] = o2.reshape(C, 2 * P)[:cnt]
    return out


# revision 29
# speedup vs baseline: 1.0299x; 1.0299x over previous
"""MultiHeadDecoder (moe_routing) Trainium2 kernel, v17.

Expert-parallel: each of 8 cores owns one head. Host groups samples by
head, pads to capacity C (multiple of 8), ships everything bf16 (PSUM
accumulates f32; tolerance 2e-2 vs bf16 wire error ~4e-3; fp8/DoubleRow
was simulated exactly and FAILS the gate at 3.4e-2).

Both stages keep weights stationary in the PE and stream sample columns:
  stage A:  ht[hc][hid,s]  = relu(sum_k W1[k,hc]^T @ X^T[k][:,s] + b1)
  stage B:  outT[of][of,s] = sum_hc W2[of,hc]^T @ ht[hc][:,s] + b2
Output is transposed ([out_feature, sample]); host untransposes.

Timing model (measured): fixed NEFF preamble ~7us; dynamic DMA rings'
first packet ~8.1-8.7/9.3-9.4/10.4us (sync/scalar/gpsimd); dense DRAM
input regions move ~8-10ns/packet vs ~15 strided; ~0.9us completion-sem
latency after a DMA's last packet; teardown (checks + barrier +
semaphore zero-storm) ~2.7-3.6us, scaling weakly with DMA count.
PE HAM: 1.2GHz cold, un-throttles after ~3.4-6.8us of sustained PE
busy (free-running window: the dominant +-1.5us run-to-run variance);
warm MM gap = N/2.4 + 2.5ns (112ns at N=264); any PE-idle hole before
the first xin-gated matmul resets the window (measured +2.6us).

Schedule: warmup matmuls bridge program start to xin arrival with no
hole.  xin ships as two dense halves (k0+W1k0+biases on sync, k1+W1k1
on scalar) so relu never gates on the tail; w2 as eight dense 2-of-tile
blocks across the rings, each landing >=2us before its stage-B need.
Stage A is k-major with k1 trailing one hc behind k0, so hc0's psum
closes right after the k1 half lands and the relu chain (g0->DVE,
g1->ACT, per-hc, ~2us) starts early.  One 8-bank PSUM pool serves
warmup + stage A + stage B: stage B holds 4 of-tiles of headroom, so
the psum-reuse convoy (of n+2 gated on of n's bias) never binds.
of0/of1 interleave hc-major to absorb the relu chain once.  Outputs
ride the sync ring as of-pairs (keeps it hot); the endgame fans out:
of14 on gpsimd, of15's g-halves on scalar+sync, so the final transfers
issue and stream in parallel.
"""

import numpy as np

import concourse.bass as bass
import concourse.mybir as mybir
from concourse import bacc
from concourse.tile import TileContext
from concourse.bass_utils import run_bass_kernel_spmd

IN_F, HID, OUT_F, N_HEADS, BATCH = 256, 512, 2048, 8, 4096
N_CORES = 8
P = 128
KI = IN_F // P      # 2 input-feature chunks
HC = HID // P       # 4 hidden chunks
OF = OUT_F // P     # 16 output-feature tiles
NB = HC + OF        # bias cols (b1: 4, b2: 16)

f32 = mybir.dt.float32
bf16 = mybir.dt.bfloat16

try:
    from ml_dtypes import bfloat16 as np_bf16
except ImportError:
    import jax.numpy as jnp
    np_bf16 = jnp.bfloat16

_NC_CACHE: dict = {}

# Warmups must bridge the PE seamlessly from program start (~6.8-7.4us)
# to the first xin-gated matmul (~10.3-11us): any PE-idle hole before the
# real work resets the HAM activity window and the whole of stage A runs
# at 1.2GHz (measured +2.6us on a WARM_PRE=10 run with a 1.2us hole).
WARM_PRE = 16    # 264-col warmups bridging program start to xin arrival


def build_nc(C: int):
    """Per-core Bass program for sample capacity C (multiple of 8)."""
    G = C // 2
    assert G <= 512
    H0 = C + HID + NB    # cols in xin half 0: X^T k0 | W1 k0 | b1 | b2
    H1 = C + HID         # cols in xin half 1: X^T k1 | W1 k1
    NIN = H0 + H1
    W2C = HC * P         # 512 w2 cols per of-tile

    nc = bacc.Bacc("TRN2", target_bir_lowering=False, debug=False,
                   num_devices=N_CORES)
    # Inputs are laid out so every DMA reads a fully dense DRAM region
    # (consecutive partition rows adjacent), letting the DMA engines
    # coalesce packets instead of moving one strided line per packet.
    xin0 = nc.dram_tensor("xin0", [P, H0], bf16, kind="ExternalInput")
    xin1 = nc.dram_tensor("xin1", [P, H1], bf16, kind="ExternalInput")
    w2 = nc.dram_tensor("w2", [8, P, 2 * W2C], bf16, kind="ExternalInput")
    outT = nc.dram_tensor("outT", [P, 14 * C], bf16, kind="ExternalOutput")
    # of14/of15 get their own contiguous [P*C] regions: consecutive
    # partitions are adjacent in DRAM, so the endgame DMAs coalesce into
    # fat packets (the strided [P, OF*C] layout moves 1KB lines).
    outT2 = nc.dram_tensor("outT2", [2, P, C], bf16, kind="ExternalOutput")
    # Scratch sink for a tiny ring-warmer DMA in the endgame (host
    # ignores it): keeps the sync ring's descriptor engine streaming
    # between the of12-13 pair and the final of15 transfer, which
    # otherwise restarts ~1.4us slow after a ~1.2us idle.
    warmo = nc.dram_tensor("warmo", [P, 16], bf16, kind="ExternalOutput")

    relu_f = mybir.ActivationFunctionType.Relu
    ident = mybir.ActivationFunctionType.Identity
    op_add = mybir.AluOpType.add
    op_max = mybir.AluOpType.max

    with TileContext(nc) as tc:
        with (
            tc.tile_pool(name="const", bufs=1) as const,
            tc.tile_pool(name="psum", bufs=8, space="PSUM") as psum,
            tc.tile_pool(name="outp", bufs=3) as outp,
            tc.tile_pool(name="outt", bufs=2) as outt,
        ):
            # Warmup matmuls on an uninitialized tile (values irrelevant).
            wsrc = const.tile([P, max(264, G)], bf16, tag="warm")
            nc.gpsimd.memset(wsrc[:, :1], 0.0)
            wps = psum.tile([P, G], f32, tag="ps", name="warmps")
            for i in range(WARM_PRE):
                nc.tensor.matmul(wps[:], lhsT=wsrc[:, :P],
                                 rhs=wsrc[:, :G], start=True, stop=True)

            # --- input DMAs ---
            xs = const.tile([P, NIN], bf16, tag="xin")
            w2s = const.tile([P, OF * W2C], bf16, tag="w2s")
            HB = 2 * W2C

            nc.sync.dma_start(xs[:, :H0], xin0[:])
            nc.scalar.dma_start(xs[:, H0:], xin1[:])
            # w2 ships as 2-of-tile blocks, each with its own completion
            # semaphore, spread over the three rings so every block lands
            # well before its stage-B need (a fused of4-7 block was
            # observed completing only ~18us on a cold-HAM run, starving
            # of4's ldweights for ~0.9us).
            w2_order = [(nc.gpsimd, 0), (nc.sync, 1), (nc.scalar, 2),
                        (nc.scalar, 3), (nc.gpsimd, 4), (nc.gpsimd, 5),
                        (nc.sync, 6), (nc.sync, 7)]
            for eng, blk in w2_order:
                eng.dma_start(w2s[:, blk * HB:(blk + 1) * HB], w2[blk])

            def xt_cols(k, g):
                base = (0 if k == 0 else H0) + g * G
                return xs[:, base:base + G]

            def w1_tile(k, hc):
                base = (0 if k == 0 else H0) + C + hc * P
                return xs[:, base:base + P]

            def w2_tile(of, hc):
                b = of * W2C + hc * P
                return w2s[:, b:b + P]
            # (w2s columns are of-major: of*512 + hc*128 + oc)

            # biases ship bf16 at the tail of xin half 0; convert to f32
            bconv = const.tile([P, NB], f32, tag="bconv")
            nc.gpsimd.tensor_scalar_add(bconv[:], xs[:, C + HID:H0], 0.0)
            b1_s = bconv[:, 0:HC]
            b2_s = bconv[:, HC:NB]

            # --- stage A: ht[hc] = relu(X @ W1 + b1)^T ---
            # Full k-major (8 concurrent accumulators, 7 pool slots + the
            # warmup slot): all k0 matmuls run before the k1 xin half
            # lands.  relu fans g0->DVE / g1->ACT in hc production order
            # (gpsimd cannot read PSUM).
            hts = [const.tile([P, C], bf16, tag=f"ht{hc}", name=f"ht{hc}")
                   for hc in range(HC)]
            pssA = {(hc, g): psum.tile([P, G], f32, tag="ps",
                                       name=f"psA{hc}_{g}")
                    for hc in range(HC) for g in range(2)}
            # k1 trails the k0 stream by one hc so hc0's accumulation
            # finishes (and its relu starts) ~0.5us after the k1 xin
            # half lands, instead of after the whole k0 sweep.
            for k, hc in [(0, 0), (0, 1), (1, 0), (0, 2),
                          (1, 1), (0, 3), (1, 2), (1, 3)]:
                for g in range(2):
                    nc.tensor.matmul(
                        pssA[hc, g][:],
                        lhsT=w1_tile(k, hc),
                        rhs=xt_cols(k, g),
                        start=(k == 0), stop=(k == KI - 1),
                    )
            for hc in range(HC):
                nc.vector.tensor_scalar(
                    hts[hc][:, 0:G], pssA[hc, 0][:],
                    b1_s[:, hc:hc + 1], 0.0, op_add, op_max,
                )
                nc.scalar.activation(hts[hc][:, G:C], pssA[hc, 1][:],
                                     relu_f, bias=b1_s[:, hc:hc + 1])

            # --- stage B: outT[of] = (H @ W2 + b2)^T, bf16 ---
            # of0/of1 interleave hc-major: both are paced by the same relu
            # chain, so running them in lockstep absorbs the chain once.
            # Outputs: of-pairs (and two final singles) all on the sync
            # ring so it stays hot into the endgame.
            def bias_out(pss, of, dst_t, off):
                for g in range(2):
                    dst = dst_t[:, off + g * G:off + (g + 1) * G]
                    if g == 1:
                        nc.scalar.activation(dst, pss[g][:], ident,
                                             bias=b2_s[:, of:of + 1])
                    else:
                        nc.vector.tensor_scalar_add(dst, pss[g][:],
                                                    b2_s[:, of:of + 1])

            ot = outp.tile([P, 2 * C], bf16, tag="op")
            pss01 = [[psum.tile([P, G], f32, tag="ps", name=f"psB{of}_{g}")
                      for g in range(2)] for of in range(2)]
            for hc in range(HC):
                for of in range(2):
                    for g in range(2):
                        nc.tensor.matmul(
                            pss01[of][g][:],
                            lhsT=w2_tile(of, hc),
                            rhs=hts[hc][:, g * G:(g + 1) * G],
                            start=(hc == 0), stop=(hc == HC - 1),
                        )
            bias_out(pss01[0], 0, ot, 0)
            bias_out(pss01[1], 1, ot, C)
            nc.sync.dma_start(outT[:, 0:2 * C], ot[:])

            o14 = o15 = None
            for of in range(2, OF):
                pss = [psum.tile([P, G], f32, tag="ps", name=f"psB{of}_{g}")
                       for g in range(2)]
                for hc in range(HC):
                    for g in range(2):
                        nc.tensor.matmul(
                            pss[g][:],
                            lhsT=w2_tile(of, hc),
                            rhs=hts[hc][:, g * G:(g + 1) * G],
                            start=(hc == 0), stop=(hc == HC - 1),
                        )
                if of < 14:
                    if of % 2 == 0:
                        ot = outp.tile([P, 2 * C], bf16, tag="op")
                    dst_t, off = ot, (of % 2) * C
                elif of == 14:
                    o14 = outt.tile([P, C], bf16, tag="o14")
                    dst_t, off = o14, 0
                else:
                    o15 = outt.tile([P, C], bf16, tag="o15")
                    dst_t, off = o15, 0
                bias_out(pss, of, dst_t, off)
                if of < 14 and of % 2 == 1:
                    pair = of // 2
                    nc.sync.dma_start(
                        outT[:, 2 * pair * C:(2 * pair + 2) * C], ot[:])
                elif of == 14:
                    # Endgame drain fans out over three engines/rings so
                    # the final transfers issue and stream in parallel
                    # instead of serializing behind one engine's ~0.6us
                    # dma_start cost each.  The sync ring also gets a
                    # tiny warmer gated on of14's bias so of15's final
                    # half finds it streaming.
                    nc.gpsimd.dma_start(outT2[0], o14[:])
                    nc.sync.dma_start(warmo[:], o14[:, :16])
                elif of == 15:
                    nc.scalar.dma_start(outT2[1][:, :G], o15[:, :G])
                    nc.sync.dma_start(outT2[1][:, G:], o15[:, G:])

    nc.compile()
    return nc


def kernel(X, X_head_idx, W1, b1, W2, b2):
    X = np.ascontiguousarray(np.asarray(X, dtype=np.float32))
    idx = np.asarray(X_head_idx).astype(np.int64)
    W1 = np.asarray(W1, dtype=np.float32)
    b1 = np.asarray(b1, dtype=np.float32)
    W2 = np.asarray(W2, dtype=np.float32)
    b2 = np.asarray(b2, dtype=np.float32)

    batch = X.shape[0]
    counts = np.bincount(idx, minlength=N_HEADS)
    order = np.argsort(idx, kind="stable")
    positions = np.split(order, np.cumsum(counts)[:-1])

    C = max(64, int(-(-int(counts.max()) // 8)) * 8)
    if C not in _NC_CACHE:
        _NC_CACHE[C] = build_nc(C)
    nc = _NC_CACHE[C]

    H0 = C + HID + NB
    H1 = C + HID
    NIN = H0 + H1

    in_maps = []
    for h in range(N_HEADS):
        pos = positions[h]
        cnt = len(pos)
        xinf = np.zeros((P, NIN), dtype=np.float32)
        w1r = W1[h].reshape(KI, P, HID)
        if cnt:
            xk = X[pos].T.reshape(KI, P, cnt)  # [k, p, s]
            xinf[:, 0:cnt] = xk[0]
            xinf[:, H0:H0 + cnt] = xk[1]
        xinf[:, C:C + HID] = w1r[0]
        xinf[:, H0 + C:H0 + C + HID] = w1r[1]
        xinf[:, C + HID:C + HID + HC] = b1[h].reshape(HC, P).T
        xinf[:, C + HID + HC:H0] = b2[h].reshape(OF, P).T
        # w2 packed: [p, of*512 + hc*128 + oc] = W2[hc*128+p, of*128+oc],
        # then split into 8 dense [P, 1024]-col blocks of 2 of-tiles.
        w2r = W2[h].reshape(HC, P, OF, P)              # [hc, p, of, oc]
        w2p = np.ascontiguousarray(np.transpose(w2r, (1, 2, 0, 3)))
        w2p = w2p.reshape(P, 8, 2 * HC * P).transpose(1, 0, 2)
        in_maps.append({
            "xin0": xinf[:, :H0].astype(np_bf16),
            "xin1": xinf[:, H0:].astype(np_bf16),
            "w2": np.ascontiguousarray(w2p).astype(np_bf16),
        })

    try:
        res = run_bass_kernel_spmd(nc, in_maps, list(range(N_CORES)))
    except Exception:
        res = run_bass_kernel_spmd(nc, in_maps, list(range(N_CORES)))

    out = np.empty((batch, OUT_F), dtype=np.float32)
    for h in range(N_HEADS):
        pos = positions[h]
        cnt = len(pos)
        if cnt:
            o = np.asarray(res.results[h]["outT"]).astype(np.float32)
            o = o.reshape(P, 14, C)                    # [p, of, s]
            o = np.transpose(o, (2, 1, 0))             # [s, of, p]
            out[pos, :14 * P] = o.reshape(C, 14 * P)[:cnt]
            o2 = np.asarray(res.results[h]["outT2"]).astype(np.float32)
            o2 = np.transpose(o2, (2, 0, 1))           # [s, of, p]
            out[pos, 14 * P:] = o2.reshape(C, 2 * P)[:cnt]
    return out
